# revision 9
# baseline (speedup 1.0000x reference)
"""Trainium2 Bass kernel for nn_BasicTransformer (B=4, T=1024, C=H=768,
vocab 50257, single-head causal attention + LM head).

Sharding: 8 cores = 4 batches x 2 vocab halves. Each core computes the
full embedding+attention for its batch (duplicated across the vocab pair)
and the LM-head matmul for its vocab half.

The LM head (~95% of FLOPs) runs in fp8 e4m3 with perf_mode=DoubleRow
(2 fp8 weights per PE cell, 256-deep contraction per instruction) for
tokens >= 128.  Early tokens average few v-rows, so their attention
output (and hence their logits, which include the global max) is large;
fp8's ~5% relative error there breaks the 2e-2 gate.  A cheap bf16
correction pass recomputes logits for tokens 0..127 in transposed layout
(stationary att-out columns, moving bf16 W_lm; 1/8 of the tokens at
bf16 rate) and the host overlays it.  q/k projections and attention
scores are also fp8 DoubleRow (softmax washes out score noise); the
v/probs path stays bf16 because early-token logits inherit its error
linearly.

W_lm streams from HBM once, in bf16; the fp8 copy for the DoubleRow
matmuls is derived per-chunk on the otherwise-idle GpSimd engine, saving
~19 MB of HBM traffic per core (DMA is the co-bottleneck).  The W stream
is software-pipelined 3 chunks deep starting before the embedding phase.
Power-of-2 pre-scales keep fp8 values in e4m3's happy range and fold
into activation-copy constants (zero extra ops).
"""

import numpy as np

import concourse.bass as bass
import concourse.mybir as mybir
import concourse.tile as tile
from concourse import bacc, bass_utils
from concourse.masks import make_causal_mask, make_identity

P = 128
T = 1024          # sequence length
C = 768           # features == head size
KC = C // P       # 6 contraction chunks
KD = KC // 2      # 3 double-row contraction chunks
TT = T // P       # 8 token tiles
VOCAB = 50257
VH = 25216        # padded vocab half (= 197 * 128), 2*VH >= VOCAB
VT = VH // P      # 197 vocab tiles per core
SCALE = float(C) ** -0.5
NEG = -1e30
DR = mybir.MatmulPerfMode.DoubleRow

# power-of-2 fp8 pre-scales (host folds SX into the embedding tables and
# SW/SWL into the weights; the rest fold into on-chip copy constants)
SX = 512.0     # x = embed + pos   (|x|max ~0.15 -> 78)
SW = 512.0     # Wq/Wk              (|W|max ~0.1  -> 50)
SQ = 512.0     # q, k               (|q|max ~0.09 -> 46)
SO = 512.0     # attention out      (|o|max ~0.054 -> 27)
SWL = 512.0    # W_lm

F32 = mybir.dt.float32
BF16 = mybir.dt.bfloat16
F8 = mybir.dt.float8e4
I32 = mybir.dt.int32
IDENT = mybir.ActivationFunctionType.Identity

_CACHE = {}


def _build(with_bias):
    nc = bacc.Bacc("TRN2", target_bir_lowering=False, debug=False)

    idx_d = nc.dram_tensor("idx", [T], I32, kind="ExternalInput")
    wemb_d = nc.dram_tensor("W_embed", [VOCAB, C], BF16, kind="ExternalInput")
    wpos_d = nc.dram_tensor("W_pos", [T, C], BF16, kind="ExternalInput")
    wq_d = nc.dram_tensor("Wq", [C, C], F8, kind="ExternalInput")
    wk_d = nc.dram_tensor("Wk", [C, C], F8, kind="ExternalInput")
    wv_d = nc.dram_tensor("Wv", [C, C], BF16, kind="ExternalInput")
    wlmb_d = nc.dram_tensor("W_lm_bf", [C, VH], BF16, kind="ExternalInput")
    blm_d = nc.dram_tensor("b_lm", [VH], F32, kind="ExternalInput")
    out_d = nc.dram_tensor("logitsT", [VH, T], BF16, kind="ExternalOutput")
    oute_d = nc.dram_tensor("logitsE", [P, VH], BF16, kind="ExternalOutput")

    with tile.TileContext(nc) as tc:
        _body(tc, nc, idx_d, wemb_d, wpos_d, wq_d, wk_d, wv_d,
              wlmb_d, blm_d, out_d, oute_d, with_bias)
    nc.compile()
    return nc


def _body(tc, nc, idx_d, wemb_d, wpos_d, wq_d, wk_d, wv_d, wlmb_d,
          blm_d, out_d, oute_d, with_bias):
    from contextlib import ExitStack

    CHUNK = 1024  # vocab columns per W_lm DMA chunk; VH = 24*1024 + 640
    offs = list(range(0, VH, CHUNK))
    WPIPE = 3     # W_lm chunk prefetch depth

    with ExitStack() as ctx:
        const = ctx.enter_context(tc.tile_pool(name="const", bufs=1))

        ident = const.tile([P, P], F32)
        make_identity(nc, ident[:])
        cmask = const.tile([P, P], F32)
        make_causal_mask(nc, cmask[:], mask_val=NEG)
        ident_bf = const.tile([P, P], BF16)
        nc.vector.tensor_copy(ident_bf[:], ident[:])
        blm_s = const.tile([P, VT], F32)

        # LM-head streaming pools, open from the start so the W_lm stream
        # can run during the whole preamble
        ph5b = ctx.enter_context(tc.tile_pool(name="ph5b", bufs=WPIPE))
        ph58 = ctx.enter_context(tc.tile_pool(name="ph58", bufs=WPIPE))
        out5 = ctx.enter_context(tc.tile_pool(name="out5", bufs=6))
        oute5 = ctx.enter_context(tc.tile_pool(name="oute5", bufs=3))

        att_ctx = ExitStack()
        attp = att_ctx.enter_context(tc.tile_pool(name="attp", bufs=1))
        qT = attp.tile([P, KC, T], F8)        # q transposed [h, t] * SQ
        kT = attp.tile([P, KC, T], F8)        # k transposed [h, t] * SQ
        v_s = attp.tile([P, TT, C], BF16)     # v            [s, h]
        # att output transposed [h, t] * SO, in h-chunk pairs for DoubleRow
        oT = [const.tile([P, 2, T], F8, name=f"oT{hp}") for hp in range(KD)]
        # bf16 att output for tokens 0..127 (early correction pass)
        oTe = const.tile([P, KC, P], BF16)

        wlbs = {}

        def issue_w(i):
            if i >= len(offs):
                return
            off = offs[i]
            w = min(CHUNK, VH - off)
            wlb = ph5b.tile([P, KC, CHUNK], BF16, tag="wlmb", name=f"wlb{i}")
            nc.scalar.dma_start(
                wlb[:, :, :w],
                wlmb_d.ap()[:, off:off + w].rearrange("(k p) n -> p k n", p=P))
            wlbs[i] = wlb

        early_ctx = ExitStack()
        early = early_ctx.enter_context(tc.tile_pool(name="early", bufs=1))
        xT = early.tile([P, KC, T], BF16)     # x transposed [c, t] * SX
        xT8 = early.tile([P, KC, T], F8)      # fp8 copy for q/k projections

        # ---- phase 1: embedding gather + positional add + transpose ----
        with tc.tile_pool(name="ph1", bufs=1) as ph1, \
             tc.tile_pool(name="ps1", bufs=4, space="PSUM") as ps1:
            idx_ts, xgs, xps = [], [], []
            for t in range(TT):
                idx_t = ph1.tile([P, 1], I32, tag=f"idx{t}")
                nc.sync.dma_start(idx_t[:], idx_d.ap()[t * P:(t + 1) * P, None])
                idx_ts.append(idx_t)
                xp = ph1.tile([P, C], BF16, tag=f"xp{t}")
                nc.scalar.dma_start(xp[:], wpos_d.ap()[t * P:(t + 1) * P, :])
                xps.append(xp)
                xg = ph1.tile([P, C], BF16, tag=f"xg{t}")
                nc.gpsimd.indirect_dma_start(
                    out=xg[:], out_offset=None, in_=wemb_d.ap()[:],
                    in_offset=bass.IndirectOffsetOnAxis(ap=idx_ts[t][:, :1], axis=0))
                xgs.append(xg)
            wq_s = early.tile([P, KC, C], F8)
            wk_s = early.tile([P, KC, C], F8)
            wv_s = early.tile([P, KC, C], BF16)
            for k in range(KC):
                for (w_s, w_d) in ((wq_s, wq_d), (wk_s, wk_d), (wv_s, wv_d)):
                    nc.sync.dma_start(
                        w_s[:, k],
                        w_d.ap()[k * P:(k + 1) * P, :])
            nc.sync.dma_start(blm_s[:], blm_d.ap().rearrange("(o p) -> p o", p=P))
            # start the W_lm stream behind the phase-1 DMAs
            for i in range(WPIPE):
                issue_w(i)
            for t in range(TT):
                xg, xp = xgs[t], xps[t]
                xb = ph1.tile([P, C], BF16, tag=f"xb{t}")
                nc.vector.tensor_add(out=xb[:], in0=xg[:], in1=xp[:])
                for k in range(KC):
                    tp = ps1.tile([P, P], BF16, tag="tp")
                    nc.tensor.transpose(
                        tp[:], xb[:, k * P:(k + 1) * P], ident_bf[:])
                    nc.vector.tensor_copy(xT[:, k, t * P:(t + 1) * P], tp[:])
                    nc.scalar.copy(xT8[:, k, t * P:(t + 1) * P], tp[:])

        # ---- phase 2: q/k (fp8 DoubleRow) and v (bf16) projections ----
        QSC = SQ / (SX * SW)   # psum q = q*SX*SW -> store q*SQ
        VSC = 1.0 / SX         # psum v = v*SX    -> store v
        with tc.tile_pool(name="ps2", bufs=6, space="PSUM") as ps2:
            for half in range(2):
                for (w_s, dstT) in ((wq_s, qT), (wk_s, kT)):
                    for h in range(KC):
                        pt = ps2.tile([P, 512], F32, tag="qk")
                        for k in range(KD):
                            nc.tensor.matmul(
                                pt[:],
                                w_s[:, 2 * k:2 * k + 2, h * P:(h + 1) * P],
                                xT8[:, 2 * k:2 * k + 2, half * 512:(half + 1) * 512],
                                start=(k == 0), stop=(k == KD - 1), perf_mode=DR)
                        nc.scalar.activation(
                            dstT[:, h, half * 512:(half + 1) * 512], pt[:],
                            IDENT, scale=QSC)
            for s in range(TT):
                for (n0, n1) in ((0, 512), (512, 768)):
                    pt = ps2.tile([P, 512], F32, tag="qk")
                    for k in range(KC):
                        nc.tensor.matmul(
                            pt[:, :n1 - n0],
                            xT[:, k, s * P:(s + 1) * P],
                            wv_s[:, k, n0:n1],
                            start=(k == 0), stop=(k == KC - 1))
                    nc.scalar.activation(
                        v_s[:, s, n0:n1], pt[:, :n1 - n0], IDENT, scale=VSC)
        early_ctx.close()

        pT_ctx = ExitStack()
        pTp = pT_ctx.enter_context(tc.tile_pool(name="pTp", bufs=1))
        # attention probs transposed [s, t], one tile per s-chunk
        pT = [pTp.tile([P, T], BF16, tag=f"pT{s}", name=f"pT{s}")
              for s in range(TT)]

        # zero the strictly-upper (future) blocks of pT that phase-4 matmuls
        # will read but phase 3 never writes
        for s in range(1, TT):
            lo = 0 if s < 4 else 512
            if s * P > lo:
                nc.vector.memset(pT[s][:, lo:s * P].bitcast(mybir.dt.uint16), 0)

        # ---- phase 3: causal attention rows -> pT (fp8 DoubleRow scores) ----
        ESC = SCALE / (SQ * SQ)  # psum scores = score*SQ^2
        with tc.tile_pool(name="ph3", bufs=3) as ph3, \
             tc.tile_pool(name="ps3", bufs=4, space="PSUM") as ps3:
            for t in range(TT):
                L = (t + 1) * P
                srow = ph3.tile([P, T], F32, tag="srow")
                for b0 in range(0, L, 512):
                    n = min(512, L - b0)
                    pt = ps3.tile([P, 512], F32, tag="sc")
                    for k in range(KD):
                        nc.tensor.matmul(
                            pt[:, :n],
                            qT[:, 2 * k:2 * k + 2, t * P:(t + 1) * P],
                            kT[:, 2 * k:2 * k + 2, b0:b0 + n],
                            start=(k == 0), stop=(k == KD - 1), perf_mode=DR)
                    nc.scalar.copy(srow[:, b0:b0 + n], pt[:, :n])
                # causal mask on the diagonal block
                nc.vector.tensor_add(
                    out=srow[:, t * P:(t + 1) * P],
                    in0=srow[:, t * P:(t + 1) * P], in1=cmask[:])
                nmax = ph3.tile([P, 1], F32, tag="nmax")
                nc.vector.tensor_reduce(
                    nmax[:], srow[:, :L], axis=mybir.AxisListType.X,
                    op=mybir.AluOpType.max, negate=True)
                nbias = ph3.tile([P, 1], F32, tag="nbias")
                nc.vector.tensor_scalar_mul(nbias[:], nmax[:], ESC)
                prow = ph3.tile([P, T], BF16, tag="prow")
                rsum = ph3.tile([P, 1], F32, tag="rsum")
                nc.scalar.activation(
                    prow[:, :L], srow[:, :L], mybir.ActivationFunctionType.Exp,
                    bias=nbias[:, :1], scale=ESC, accum_out=rsum[:, :1])
                rinv = ph3.tile([P, 1], F32, tag="rinv")
                nc.vector.reciprocal(rinv[:], rsum[:])
                nc.vector.tensor_scalar_mul(prow[:, :L], prow[:, :L], rinv[:, :1])
                for s in range(t + 1):
                    tp = ps3.tile([P, P], BF16, tag="tp")
                    nc.tensor.transpose(
                        tp[:], prow[:, s * P:(s + 1) * P], ident_bf[:])
                    nc.vector.tensor_copy(pT[s][:, t * P:(t + 1) * P], tp[:])

        # ---- phase 4: att_out = P @ v (bf16) ----
        with tc.tile_pool(name="ps4", bufs=4, space="PSUM") as ps4:
            for h in range(KC):
                for blk in range(2):
                    smax = 4 if blk == 0 else TT
                    pt = ps4.tile([P, 512], F32, tag="av")
                    for s in range(smax):
                        nc.tensor.matmul(
                            pt[:],
                            v_s[:, s, h * P:(h + 1) * P],
                            pT[s][:, blk * 512:(blk + 1) * 512],
                            start=(s == 0), stop=(s == smax - 1))
                    nc.scalar.activation(
                        oT[h // 2][:, h % 2, blk * 512:(blk + 1) * 512], pt[:],
                        IDENT, scale=SO)
                    if blk == 0:
                        nc.vector.tensor_copy(oTe[:, h, :], pt[:, :P])

        # ---- phase 5: LM head ----
        # main pass: fp8 DoubleRow, logitsT[v, t] for t in 128..1023
        # early pass: bf16, logitsE[t, v] for t in 0..127 (host overlays)
        LSC = 1.0 / (SO * SWL)
        ESC5 = 1.0 / SWL
        with tc.tile_pool(name="ps5", bufs=6, space="PSUM") as ps5, \
             tc.tile_pool(name="ps5e", bufs=2, space="PSUM") as ps5e:
            for i, off in enumerate(offs):
                w = min(CHUNK, VH - off)
                wlb = wlbs.pop(i)
                # fp8 copy of this W chunk, built on the idle GpSimd engine
                wl = ph58.tile([P, KC, CHUNK], F8, tag="wlm", name=f"wl{i}")
                nc.gpsimd.tensor_copy(wl[:, :, :w], wlb[:, :, :w])
                issue_w(i + WPIPE)
                for j in range(w // P):
                    vt = (off + j * P) // P
                    lo = out5.tile([P, T], BF16, tag="lo")
                    pts = [ps5.tile([P, 512], F32, tag="lp", name=f"lp{hf}")
                           for hf in range(2)]
                    for k in range(KD):
                        nc.tensor.matmul(
                            pts[0][:, :384],
                            wl[:, 2 * k:2 * k + 2, j * P:(j + 1) * P],
                            oT[k][:, 0:2, P:512],
                            start=(k == 0), stop=(k == KD - 1), perf_mode=DR)
                        nc.tensor.matmul(
                            pts[1][:],
                            wl[:, 2 * k:2 * k + 2, j * P:(j + 1) * P],
                            oT[k][:, 0:2, 512:1024],
                            start=(k == 0), stop=(k == KD - 1), perf_mode=DR)
                    if with_bias:
                        nc.scalar.activation(
                            lo[:, P:512], pts[0][:, :384], IDENT,
                            bias=blm_s[:, vt:vt + 1], scale=LSC)
                        nc.scalar.activation(
                            lo[:, 512:1024], pts[1][:], IDENT,
                            bias=blm_s[:, vt:vt + 1], scale=LSC)
                    else:
                        nc.vector.tensor_scalar_mul(
                            lo[:, P:512], pts[0][:, :384], LSC)
                        nc.vector.tensor_scalar_mul(
                            lo[:, 512:1024], pts[1][:], LSC)
                    nc.sync.dma_start(
                        out_d.ap()[vt * P:(vt + 1) * P, P:], lo[:, P:])
                # early correction pass for this vocab chunk
                for vs in range(0, w, 512):
                    n = min(512, w - vs)
                    pte = ps5e.tile([P, 512], F32, tag="lpe")
                    for k in range(KC):
                        nc.tensor.matmul(
                            pte[:, :n], oTe[:, k, :], wlb[:, k, vs:vs + n],
                            start=(k == 0), stop=(k == KC - 1))
                    loe = oute5.tile([P, 512], BF16, tag="loe")
                    nc.vector.tensor_scalar_mul(loe[:, :n], pte[:, :n], ESC5)
                    nc.sync.dma_start(
                        oute_d.ap()[:, off + vs:off + vs + n], loe[:, :n])
        pT_ctx.close()
        att_ctx.close()


def _get_nc(with_bias):
    key = ("nc", with_bias)
    if key not in _CACHE:
        _CACHE[key] = _build(with_bias)
    return _CACHE[key]


def _to_f8(x, scale):
    import ml_dtypes
    y = np.clip(np.asarray(x, np.float32) * np.float32(scale), -240.0, 240.0)
    return np.ascontiguousarray(y.astype(ml_dtypes.float8_e4m3))


def _make_in_maps(idx, W_embed, W_pos, Wq, Wk, Wv, W_lm, b_lm):
    import ml_dtypes

    W_embed = np.ascontiguousarray(
        (np.asarray(W_embed, np.float32) * np.float32(SX)).astype(
            ml_dtypes.bfloat16))
    W_pos = np.ascontiguousarray(
        (np.asarray(W_pos, np.float32) * np.float32(SX)).astype(
            ml_dtypes.bfloat16))
    wq8 = _to_f8(Wq, SW)
    wk8 = _to_f8(Wk, SW)
    wvb = np.ascontiguousarray(np.asarray(Wv).astype(ml_dtypes.bfloat16))
    halves_wb = []
    halves_b = []
    for h in range(2):
        lo = h * VH
        hi = min(VOCAB, lo + VH)
        wlb = np.zeros((C, VH), dtype=ml_dtypes.bfloat16)
        wlb[:, :hi - lo] = (np.asarray(W_lm[:, lo:hi], np.float32)
                            * np.float32(SWL)).astype(ml_dtypes.bfloat16)
        bl = np.zeros((VH,), dtype=np.float32)
        bl[:hi - lo] = b_lm[lo:hi]
        halves_wb.append(wlb)
        halves_b.append(bl)
    in_maps = []
    for core in range(8):
        b = core >> 1
        h = core & 1
        in_maps.append({
            "idx": np.ascontiguousarray(idx[b], dtype=np.int32),
            "W_embed": W_embed,
            "W_pos": W_pos,
            "Wq": wq8,
            "Wk": wk8,
            "Wv": wvb,
            "W_lm_bf": halves_wb[h],
            "b_lm": halves_b[h],
        })
    return in_maps


def _run(inputs, trace=False):
    nc = _get_nc(bool(np.any(np.asarray(inputs["b_lm"]))))
    in_maps = _make_in_maps(**inputs)
    res = bass_utils.run_bass_kernel_spmd(
        nc, in_maps, core_ids=list(range(8)), trace=trace)
    B = inputs["idx"].shape[0]
    b_lm = np.asarray(inputs["b_lm"], np.float32)
    out = np.empty((B, T, VOCAB), dtype=np.float32)
    for core in range(8):
        b = core >> 1
        h = core & 1
        lo = h * VH
        hi = min(VOCAB, lo + VH)
        out[b, :, lo:hi] = res.results[core]["logitsT"][:hi - lo, :].astype(
            np.float32).T
        out[b, :P, lo:hi] = res.results[core]["logitsE"][:, :hi - lo].astype(
            np.float32) + b_lm[lo:hi]
    return out, res


def kernel(**inputs):
    out, _ = _run(inputs, trace=False)
    return out


# revision 13
# speedup vs baseline: 1.1560x; 1.1560x over previous
"""Trainium2 Bass kernel for nn_BasicTransformer (B=4, T=1024, C=H=768,
vocab 50257, single-head causal attention + LM head).

Sharding: 8 cores = 4 batches x 2 vocab halves. Each core computes the
full embedding+attention for its batch (duplicated across the vocab pair)
and the LM-head matmul for its vocab half.

The LM head (~95% of FLOPs) runs in fp8 e4m3 with perf_mode=DoubleRow
(2 fp8 weights per PE cell, 256-deep contraction per instruction) for
tokens >= 128.  Early tokens average few v-rows, so their attention
output (and hence their logits, which include the global max) is large;
fp8's ~5% relative error there breaks the 2e-2 gate.  A cheap bf16
correction pass recomputes logits for tokens 0..127 in transposed layout
(stationary att-out columns, moving bf16 W_lm; 1/8 of the tokens at
bf16 rate) and the host overlays it.  q/k projections and attention
scores are also fp8 DoubleRow (softmax washes out score noise); the
v/probs path stays bf16 because early-token logits inherit its error
linearly.

W_lm streams from HBM in two dtypes (fp8 for the DoubleRow pass, bf16
for the early pass), software-pipelined 3-4 chunks deep starting before
the embedding phase so the stream never starves the PE.  Power-of-2
pre-scales keep fp8 values in e4m3's happy range and fold into
activation-copy constants (zero extra ops).
"""

import numpy as np

import concourse.bass as bass
import concourse.mybir as mybir
import concourse.tile as tile
from concourse import bacc, bass_utils
from concourse.masks import make_causal_mask, make_identity

P = 128
T = 1024          # sequence length
C = 768           # features == head size
KC = C // P       # 6 contraction chunks
KD = KC // 2      # 3 double-row contraction chunks
TT = T // P       # 8 token tiles
VOCAB = 50257
VH = 25216        # padded vocab half (= 197 * 128), 2*VH >= VOCAB
VT = VH // P      # 197 vocab tiles per core
SCALE = float(C) ** -0.5
NEG = -1e30
DR = mybir.MatmulPerfMode.DoubleRow

# power-of-2 fp8 pre-scales (host folds SX into the embedding tables and
# SW/SWL into the weights; the rest fold into on-chip copy constants)
SX = 512.0     # x = embed + pos   (|x|max ~0.15 -> 78)
SW = 512.0     # Wq/Wk              (|W|max ~0.1  -> 50)
SQ = 512.0     # q, k               (|q|max ~0.09 -> 46)
SO = 512.0     # attention out      (|o|max ~0.054 -> 27)
SWL = 512.0    # W_lm

F32 = mybir.dt.float32
BF16 = mybir.dt.bfloat16
F8 = mybir.dt.float8e4
I32 = mybir.dt.int32
IDENT = mybir.ActivationFunctionType.Identity

_CACHE = {}


def _build(with_bias):
    nc = bacc.Bacc("TRN2", target_bir_lowering=False, debug=False)

    idx_d = nc.dram_tensor("idx", [T], I32, kind="ExternalInput")
    wemb_d = nc.dram_tensor("W_embed", [VOCAB, C], BF16, kind="ExternalInput")
    wpos_d = nc.dram_tensor("W_pos", [T, C], BF16, kind="ExternalInput")
    wq_d = nc.dram_tensor("Wq", [C, C], F8, kind="ExternalInput")
    wk_d = nc.dram_tensor("Wk", [C, C], F8, kind="ExternalInput")
    wv_d = nc.dram_tensor("Wv", [C, C], BF16, kind="ExternalInput")
    wlm_d = nc.dram_tensor("W_lm", [C, VH], F8, kind="ExternalInput")
    wlmb_d = nc.dram_tensor("W_lm_bf", [C, VH], BF16, kind="ExternalInput")
    blm_d = nc.dram_tensor("b_lm", [VH], F32, kind="ExternalInput")
    out_d = nc.dram_tensor("logitsT", [VH, T], BF16, kind="ExternalOutput")
    oute_d = nc.dram_tensor("logitsE", [P, VH], BF16, kind="ExternalOutput")

    with tile.TileContext(nc) as tc:
        _body(tc, nc, idx_d, wemb_d, wpos_d, wq_d, wk_d, wv_d,
              wlm_d, wlmb_d, blm_d, out_d, oute_d, with_bias)
    nc.compile()
    return nc


def _body(tc, nc, idx_d, wemb_d, wpos_d, wq_d, wk_d, wv_d, wlm_d, wlmb_d,
          blm_d, out_d, oute_d, with_bias):
    from contextlib import ExitStack

    CHUNK = 1024  # vocab columns per W_lm DMA chunk; VH = 24*1024 + 640
    offs = list(range(0, VH, CHUNK))
    WPIPE = 3     # W_lm chunk prefetch depth

    with ExitStack() as ctx:
        const = ctx.enter_context(tc.tile_pool(name="const", bufs=1))

        ident = const.tile([P, P], F32)
        make_identity(nc, ident[:])
        cmask = const.tile([P, P], F32)
        make_causal_mask(nc, cmask[:], mask_val=NEG)
        ident_bf = const.tile([P, P], BF16)
        nc.vector.tensor_copy(ident_bf[:], ident[:])
        blm_s = const.tile([P, VT], F32)

        # LM-head streaming pools, open from the start so the W_lm stream
        # can run during the whole preamble
        ph5b = ctx.enter_context(tc.tile_pool(name="ph5b", bufs=WPIPE))
        ph58 = ctx.enter_context(tc.tile_pool(name="ph58", bufs=WPIPE + 1))
        out5 = ctx.enter_context(tc.tile_pool(name="out5", bufs=6))
        oute5 = ctx.enter_context(tc.tile_pool(name="oute5", bufs=3))

        att_ctx = ExitStack()
        attp = att_ctx.enter_context(tc.tile_pool(name="attp", bufs=1))
        qT = attp.tile([P, KC, T], F8)        # q transposed [h, t] * SQ
        kT = attp.tile([P, KC, T], F8)        # k transposed [h, t] * SQ
        v_s = attp.tile([P, TT, C], BF16)     # v            [s, h]
        # att output transposed [h, t] * SO, in h-chunk pairs for DoubleRow
        oT = [const.tile([P, 2, T], F8, name=f"oT{hp}") for hp in range(KD)]
        # bf16 att output for tokens 0..127 (early correction pass)
        oTe = const.tile([P, KC, P], BF16)

        wlbs = {}
        wl8s = {}

        def issue_w(i):
            if i >= len(offs):
                return
            off = offs[i]
            w = min(CHUNK, VH - off)
            wlb = ph5b.tile([P, KC, CHUNK], BF16, tag="wlmb", name=f"wlb{i}")
            nc.scalar.dma_start(
                wlb[:, :, :w],
                wlmb_d.ap()[:, off:off + w].rearrange("(k p) n -> p k n", p=P))
            wlbs[i] = wlb
            wl = ph58.tile([P, KC, CHUNK], F8, tag="wlm", name=f"wl{i}")
            nc.scalar.dma_start(
                wl[:, :, :w],
                wlm_d.ap()[:, off:off + w].rearrange("(k p) n -> p k n", p=P))
            wl8s[i] = wl

        early_ctx = ExitStack()
        early = early_ctx.enter_context(tc.tile_pool(name="early", bufs=1))
        xT = early.tile([P, KC, T], BF16)     # x transposed [c, t] * SX
        xT8 = early.tile([P, KC, T], F8)      # fp8 copy for q/k projections

        # ---- phase 1: embedding gather + positional add + transpose ----
        with tc.tile_pool(name="ph1", bufs=1) as ph1, \
             tc.tile_pool(name="ps1", bufs=4, space="PSUM") as ps1:
            idx_ts, xgs, xps = [], [], []
            for t in range(TT):
                idx_t = ph1.tile([P, 1], I32, tag=f"idx{t}")
                nc.sync.dma_start(idx_t[:], idx_d.ap()[t * P:(t + 1) * P, None])
                idx_ts.append(idx_t)
                xp = ph1.tile([P, C], BF16, tag=f"xp{t}")
                nc.scalar.dma_start(xp[:], wpos_d.ap()[t * P:(t + 1) * P, :])
                xps.append(xp)
                xg = ph1.tile([P, C], BF16, tag=f"xg{t}")
                nc.gpsimd.indirect_dma_start(
                    out=xg[:], out_offset=None, in_=wemb_d.ap()[:],
                    in_offset=bass.IndirectOffsetOnAxis(ap=idx_ts[t][:, :1], axis=0))
                xgs.append(xg)
            wq_s = early.tile([P, KC, C], F8)
            wk_s = early.tile([P, KC, C], F8)
            wv_s = early.tile([P, KC, C], BF16)
            for k in range(KC):
                for (w_s, w_d) in ((wq_s, wq_d), (wk_s, wk_d), (wv_s, wv_d)):
                    nc.sync.dma_start(
                        w_s[:, k],
                        w_d.ap()[k * P:(k + 1) * P, :])
            nc.sync.dma_start(blm_s[:], blm_d.ap().rearrange("(o p) -> p o", p=P))
            # start the W_lm stream behind the phase-1 DMAs
            for i in range(WPIPE):
                issue_w(i)
            for t in range(TT):
                xg, xp = xgs[t], xps[t]
                xb = ph1.tile([P, C], BF16, tag=f"xb{t}")
                nc.vector.tensor_add(out=xb[:], in0=xg[:], in1=xp[:])
                for k in range(KC):
                    tp = ps1.tile([P, P], BF16, tag="tp")
                    nc.tensor.transpose(
                        tp[:], xb[:, k * P:(k + 1) * P], ident_bf[:])
                    nc.vector.tensor_copy(xT[:, k, t * P:(t + 1) * P], tp[:])
                    nc.scalar.copy(xT8[:, k, t * P:(t + 1) * P], tp[:])

        # ---- phase 2: q/k (fp8 DoubleRow) and v (bf16) projections ----
        QSC = SQ / (SX * SW)   # psum q = q*SX*SW -> store q*SQ
        VSC = 1.0 / SX         # psum v = v*SX    -> store v
        with tc.tile_pool(name="ps2", bufs=6, space="PSUM") as ps2:
            for half in range(2):
                for (w_s, dstT) in ((wq_s, qT), (wk_s, kT)):
                    for h in range(KC):
                        pt = ps2.tile([P, 512], F32, tag="qk")
                        for k in range(KD):
                            nc.tensor.matmul(
                                pt[:],
                                w_s[:, 2 * k:2 * k + 2, h * P:(h + 1) * P],
                                xT8[:, 2 * k:2 * k + 2, half * 512:(half + 1) * 512],
                                start=(k == 0), stop=(k == KD - 1), perf_mode=DR)
                        nc.scalar.activation(
                            dstT[:, h, half * 512:(half + 1) * 512], pt[:],
                            IDENT, scale=QSC)
            for s in range(TT):
                for (n0, n1) in ((0, 512), (512, 768)):
                    pt = ps2.tile([P, 512], F32, tag="qk")
                    for k in range(KC):
                        nc.tensor.matmul(
                            pt[:, :n1 - n0],
                            xT[:, k, s * P:(s + 1) * P],
                            wv_s[:, k, n0:n1],
                            start=(k == 0), stop=(k == KC - 1))
                    nc.scalar.activation(
                        v_s[:, s, n0:n1], pt[:, :n1 - n0], IDENT, scale=VSC)
        early_ctx.close()

        pT_ctx = ExitStack()
        pTp = pT_ctx.enter_context(tc.tile_pool(name="pTp", bufs=1))
        # attention probs transposed [s, t], one tile per s-chunk
        pT = [pTp.tile([P, T], BF16, tag=f"pT{s}", name=f"pT{s}")
              for s in range(TT)]

        # zero the strictly-upper (future) blocks of pT that phase-4 matmuls
        # will read but phase 3 never writes
        for s in range(1, TT):
            lo = 0 if s < 4 else 512
            if s * P > lo:
                nc.vector.memset(pT[s][:, lo:s * P].bitcast(mybir.dt.uint16), 0)

        # ---- phase 3: causal attention rows -> pT (fp8 DoubleRow scores) ----
        ESC = SCALE / (SQ * SQ)  # psum scores = score*SQ^2
        with tc.tile_pool(name="ph3", bufs=3) as ph3, \
             tc.tile_pool(name="ps3", bufs=4, space="PSUM") as ps3:
            for t in range(TT):
                L = (t + 1) * P
                srow = ph3.tile([P, T], F32, tag="srow")
                for b0 in range(0, L, 512):
                    n = min(512, L - b0)
                    pt = ps3.tile([P, 512], F32, tag="sc")
                    for k in range(KD):
                        nc.tensor.matmul(
                            pt[:, :n],
                            qT[:, 2 * k:2 * k + 2, t * P:(t + 1) * P],
                            kT[:, 2 * k:2 * k + 2, b0:b0 + n],
                            start=(k == 0), stop=(k == KD - 1), perf_mode=DR)
                    nc.scalar.copy(srow[:, b0:b0 + n], pt[:, :n])
                # causal mask on the diagonal block
                nc.vector.tensor_add(
                    out=srow[:, t * P:(t + 1) * P],
                    in0=srow[:, t * P:(t + 1) * P], in1=cmask[:])
                nmax = ph3.tile([P, 1], F32, tag="nmax")
                nc.vector.tensor_reduce(
                    nmax[:], srow[:, :L], axis=mybir.AxisListType.X,
                    op=mybir.AluOpType.max, negate=True)
                nbias = ph3.tile([P, 1], F32, tag="nbias")
                nc.vector.tensor_scalar_mul(nbias[:], nmax[:], ESC)
                prow = ph3.tile([P, T], BF16, tag="prow")
                rsum = ph3.tile([P, 1], F32, tag="rsum")
                nc.scalar.activation(
                    prow[:, :L], srow[:, :L], mybir.ActivationFunctionType.Exp,
                    bias=nbias[:, :1], scale=ESC, accum_out=rsum[:, :1])
                rinv = ph3.tile([P, 1], F32, tag="rinv")
                nc.vector.reciprocal(rinv[:], rsum[:])
                nc.vector.tensor_scalar_mul(prow[:, :L], prow[:, :L], rinv[:, :1])
                for s in range(t + 1):
                    tp = ps3.tile([P, P], BF16, tag="tp")
                    nc.tensor.transpose(
                        tp[:], prow[:, s * P:(s + 1) * P], ident_bf[:])
                    nc.vector.tensor_copy(pT[s][:, t * P:(t + 1) * P], tp[:])

        # ---- phase 4: att_out = P @ v (bf16) ----
        with tc.tile_pool(name="ps4", bufs=4, space="PSUM") as ps4:
            for h in range(KC):
                for blk in range(2):
                    smax = 4 if blk == 0 else TT
                    pt = ps4.tile([P, 512], F32, tag="av")
                    for s in range(smax):
                        nc.tensor.matmul(
                            pt[:],
                            v_s[:, s, h * P:(h + 1) * P],
                            pT[s][:, blk * 512:(blk + 1) * 512],
                            start=(s == 0), stop=(s == smax - 1))
                    nc.scalar.activation(
                        oT[h // 2][:, h % 2, blk * 512:(blk + 1) * 512], pt[:],
                        IDENT, scale=SO)
                    if blk == 0:
                        nc.vector.tensor_copy(oTe[:, h, :], pt[:, :P])

        # ---- phase 5: LM head ----
        # main pass: fp8 DoubleRow, logitsT[v, t] for t in 128..1023
        # early pass: bf16, logitsE[t, v] for t in 0..127 (host overlays)
        LSC = 1.0 / (SO * SWL)
        ESC5 = 1.0 / SWL
        with tc.tile_pool(name="ps5", bufs=6, space="PSUM") as ps5, \
             tc.tile_pool(name="ps5e", bufs=2, space="PSUM") as ps5e:
            for i, off in enumerate(offs):
                w = min(CHUNK, VH - off)
                wlb = wlbs.pop(i)
                wl = wl8s.pop(i)
                issue_w(i + WPIPE)
                for j in range(w // P):
                    vt = (off + j * P) // P
                    lo = out5.tile([P, T], BF16, tag="lo")
                    pts = [ps5.tile([P, 512], F32, tag="lp", name=f"lp{hf}")
                           for hf in range(2)]
                    for k in range(KD):
                        nc.tensor.matmul(
                            pts[0][:, :384],
                            wl[:, 2 * k:2 * k + 2, j * P:(j + 1) * P],
                            oT[k][:, 0:2, P:512],
                            start=(k == 0), stop=(k == KD - 1), perf_mode=DR)
                        nc.tensor.matmul(
                            pts[1][:],
                            wl[:, 2 * k:2 * k + 2, j * P:(j + 1) * P],
                            oT[k][:, 0:2, 512:1024],
                            start=(k == 0), stop=(k == KD - 1), perf_mode=DR)
                    if with_bias:
                        nc.scalar.activation(
                            lo[:, P:512], pts[0][:, :384], IDENT,
                            bias=blm_s[:, vt:vt + 1], scale=LSC)
                        nc.scalar.activation(
                            lo[:, 512:1024], pts[1][:], IDENT,
                            bias=blm_s[:, vt:vt + 1], scale=LSC)
                    else:
                        nc.vector.tensor_scalar_mul(
                            lo[:, P:512], pts[0][:, :384], LSC)
                        nc.vector.tensor_scalar_mul(
                            lo[:, 512:1024], pts[1][:], LSC)
                    nc.sync.dma_start(
                        out_d.ap()[vt * P:(vt + 1) * P, P:], lo[:, P:])
                # early correction pass for this vocab chunk
                for vs in range(0, w, 512):
                    n = min(512, w - vs)
                    pte = ps5e.tile([P, 512], F32, tag="lpe")
                    for k in range(KC):
                        nc.tensor.matmul(
                            pte[:, :n], oTe[:, k, :], wlb[:, k, vs:vs + n],
                            start=(k == 0), stop=(k == KC - 1))
                    loe = oute5.tile([P, 512], BF16, tag="loe")
                    nc.vector.tensor_scalar_mul(loe[:, :n], pte[:, :n], ESC5)
                    nc.sync.dma_start(
                        oute_d.ap()[:, off + vs:off + vs + n], loe[:, :n])
        pT_ctx.close()
        att_ctx.close()


def _get_nc(with_bias):
    key = ("nc", with_bias)
    if key not in _CACHE:
        _CACHE[key] = _build(with_bias)
    return _CACHE[key]


def _to_f8(x, scale):
    import ml_dtypes
    y = np.clip(np.asarray(x, np.float32) * np.float32(scale), -240.0, 240.0)
    return np.ascontiguousarray(y.astype(ml_dtypes.float8_e4m3))


def _make_in_maps(idx, W_embed, W_pos, Wq, Wk, Wv, W_lm, b_lm):
    import ml_dtypes

    W_embed = np.ascontiguousarray(
        (np.asarray(W_embed, np.float32) * np.float32(SX)).astype(
            ml_dtypes.bfloat16))
    W_pos = np.ascontiguousarray(
        (np.asarray(W_pos, np.float32) * np.float32(SX)).astype(
            ml_dtypes.bfloat16))
    wq8 = _to_f8(Wq, SW)
    wk8 = _to_f8(Wk, SW)
    wvb = np.ascontiguousarray(np.asarray(Wv).astype(ml_dtypes.bfloat16))
    halves_w8 = []
    halves_wb = []
    halves_b = []
    for h in range(2):
        lo = h * VH
        hi = min(VOCAB, lo + VH)
        wlb = np.zeros((C, VH), dtype=ml_dtypes.bfloat16)
        wlb[:, :hi - lo] = (np.asarray(W_lm[:, lo:hi], np.float32)
                            * np.float32(SWL)).astype(ml_dtypes.bfloat16)
        wl8 = np.zeros((C, VH), dtype=ml_dtypes.float8_e4m3)
        wl8[:, :hi - lo] = _to_f8(W_lm[:, lo:hi], SWL)
        bl = np.zeros((VH,), dtype=np.float32)
        bl[:hi - lo] = b_lm[lo:hi]
        halves_w8.append(wl8)
        halves_wb.append(wlb)
        halves_b.append(bl)
    in_maps = []
    for core in range(8):
        b = core >> 1
        h = core & 1
        in_maps.append({
            "idx": np.ascontiguousarray(idx[b], dtype=np.int32),
            "W_embed": W_embed,
            "W_pos": W_pos,
            "Wq": wq8,
            "Wk": wk8,
            "Wv": wvb,
            "W_lm": halves_w8[h],
            "W_lm_bf": halves_wb[h],
            "b_lm": halves_b[h],
        })
    return in_maps


def _run(inputs, trace=False):
    nc = _get_nc(bool(np.any(np.asarray(inputs["b_lm"]))))
    in_maps = _make_in_maps(**inputs)
    res = bass_utils.run_bass_kernel_spmd(
        nc, in_maps, core_ids=list(range(8)), trace=trace)
    B = inputs["idx"].shape[0]
    b_lm = np.asarray(inputs["b_lm"], np.float32)
    out = np.empty((B, T, VOCAB), dtype=np.float32)
    for core in range(8):
        b = core >> 1
        h = core & 1
        lo = h * VH
        hi = min(VOCAB, lo + VH)
        out[b, :, lo:hi] = res.results[core]["logitsT"][:hi - lo, :].astype(
            np.float32).T
        out[b, :P, lo:hi] = res.results[core]["logitsE"][:, :hi - lo].astype(
            np.float32) + b_lm[lo:hi]
    return out, res


def kernel(**inputs):
    out, _ = _run(inputs, trace=False)
    return out


# revision 15
# speedup vs baseline: 1.3372x; 1.1567x over previous
"""Trainium2 Bass kernel for nn_BasicTransformer (B=4, T=1024, C=H=768,
vocab 50257, single-head causal attention + LM head).

Sharding: 8 cores = 4 batches x 2 vocab halves. Each core computes the
full embedding+attention for its batch (duplicated across the vocab pair)
and the LM-head matmul for its vocab half.

The LM head (~95% of FLOPs) runs in fp8 e4m3 with perf_mode=DoubleRow
(2 fp8 weights per PE cell, 256-deep contraction per instruction) for
tokens >= 128.  Early tokens average few v-rows, so their attention
output (and hence their logits, which include the global max) is large;
fp8's ~5% relative error there breaks the 2e-2 gate.  A cheap bf16
correction pass recomputes logits for tokens 0..127 in transposed layout
(stationary att-out columns, moving bf16 W_lm; 1/8 of the tokens at
bf16 rate) and the host overlays it.  q/k projections and attention
scores are also fp8 DoubleRow (softmax washes out score noise); the
v/probs path stays bf16 because early-token logits inherit its error
linearly.

W_lm streams from HBM in two dtypes (fp8 for the DoubleRow pass, bf16
for the early pass), software-pipelined 3-4 chunks deep starting before
the embedding phase so the stream never starves the PE.  Power-of-2
pre-scales keep fp8 values in e4m3's happy range and fold into
activation-copy constants (zero extra ops).
"""

import numpy as np

import concourse.bass as bass
import concourse.mybir as mybir
import concourse.tile as tile
from concourse import bacc, bass_utils
from concourse.masks import make_causal_mask, make_identity

P = 128
T = 1024          # sequence length
C = 768           # features == head size
KC = C // P       # 6 contraction chunks
KD = KC // 2      # 3 double-row contraction chunks
TT = T // P       # 8 token tiles
VOCAB = 50257
VH = 25216        # padded vocab half (= 197 * 128), 2*VH >= VOCAB
VT = VH // P      # 197 vocab tiles per core
SCALE = float(C) ** -0.5
NEG = -1e30
DR = mybir.MatmulPerfMode.DoubleRow

# power-of-2 fp8 pre-scales (host folds SX into the embedding tables and
# SW/SWL into the weights; the rest fold into on-chip copy constants)
SX = 512.0     # x = embed + pos   (|x|max ~0.15 -> 78)
SW = 512.0     # Wq/Wk              (|W|max ~0.1  -> 50)
SQ = 512.0     # q, k               (|q|max ~0.09 -> 46)
SO = 512.0     # attention out      (|o|max ~0.054 -> 27)
SWL = 512.0    # W_lm

F32 = mybir.dt.float32
BF16 = mybir.dt.bfloat16
F8 = mybir.dt.float8e4
I32 = mybir.dt.int32
IDENT = mybir.ActivationFunctionType.Identity

_CACHE = {}


def _build(with_bias):
    nc = bacc.Bacc("TRN2", target_bir_lowering=False, debug=False)

    idx_d = nc.dram_tensor("idx", [T], I32, kind="ExternalInput")
    wemb_d = nc.dram_tensor("W_embed", [VOCAB, C], BF16, kind="ExternalInput")
    wpos_d = nc.dram_tensor("W_pos", [T, C], BF16, kind="ExternalInput")
    wq_d = nc.dram_tensor("Wq", [C, C], F8, kind="ExternalInput")
    wk_d = nc.dram_tensor("Wk", [C, C], F8, kind="ExternalInput")
    wv_d = nc.dram_tensor("Wv", [C, C], BF16, kind="ExternalInput")
    wlm_d = nc.dram_tensor("W_lm", [C, VH], F8, kind="ExternalInput")
    wlmb_d = nc.dram_tensor("W_lm_bf", [C, VH], BF16, kind="ExternalInput")
    blm_d = nc.dram_tensor("b_lm", [VH], F32, kind="ExternalInput")
    out_d = nc.dram_tensor("logitsT", [VH, T], BF16, kind="ExternalOutput")
    oute_d = nc.dram_tensor("logitsE", [P, VH], BF16, kind="ExternalOutput")

    with tile.TileContext(nc) as tc:
        _body(tc, nc, idx_d, wemb_d, wpos_d, wq_d, wk_d, wv_d,
              wlm_d, wlmb_d, blm_d, out_d, oute_d, with_bias)
    nc.compile()
    return nc


def _body(tc, nc, idx_d, wemb_d, wpos_d, wq_d, wk_d, wv_d, wlm_d, wlmb_d,
          blm_d, out_d, oute_d, with_bias):
    from contextlib import ExitStack

    CHUNK = 1024  # vocab columns per W_lm DMA chunk; VH = 24*1024 + 640
    offs = list(range(0, VH, CHUNK))
    WPIPE = 3     # W_lm chunk prefetch depth

    with ExitStack() as ctx:
        const = ctx.enter_context(tc.tile_pool(name="const", bufs=1))

        ident = const.tile([P, P], F32)
        make_identity(nc, ident[:])
        cmask = const.tile([P, P], F32)
        make_causal_mask(nc, cmask[:], mask_val=NEG)
        ident_bf = const.tile([P, P], BF16)
        nc.vector.tensor_copy(ident_bf[:], ident[:])
        blm_s = const.tile([P, VT], F32)

        # LM-head streaming pools, open from the start so the W_lm stream
        # can run during the whole preamble
        ph5b = ctx.enter_context(tc.tile_pool(name="ph5b", bufs=WPIPE))
        ph58 = ctx.enter_context(tc.tile_pool(name="ph58", bufs=WPIPE + 1))
        out5 = ctx.enter_context(tc.tile_pool(name="out5", bufs=6))
        oute5 = ctx.enter_context(tc.tile_pool(name="oute5", bufs=3))

        att_ctx = ExitStack()
        attp = att_ctx.enter_context(tc.tile_pool(name="attp", bufs=1))
        qT = attp.tile([P, KC, T], F8)        # q transposed [h, t] * SQ
        kT = attp.tile([P, KC, T], F8)        # k transposed [h, t] * SQ
        v_s = attp.tile([P, TT, C], BF16)     # v            [s, h]
        # att output transposed [h, t] * SO, in h-chunk pairs for DoubleRow
        oT = [const.tile([P, 2, T], F8, name=f"oT{hp}") for hp in range(KD)]
        # bf16 att output for tokens 0..127 (early correction pass)
        oTe = const.tile([P, KC, P], BF16)

        wlbs = {}
        wl8s = {}

        def issue_w(i):
            if i >= len(offs):
                return
            off = offs[i]
            w = min(CHUNK, VH - off)
            wlb = ph5b.tile([P, KC, CHUNK], BF16, tag="wlmb", name=f"wlb{i}")
            nc.scalar.dma_start(
                wlb[:, :, :w],
                wlmb_d.ap()[:, off:off + w].rearrange("(k p) n -> p k n", p=P))
            wlbs[i] = wlb
            wl = ph58.tile([P, KC, CHUNK], F8, tag="wlm", name=f"wl{i}")
            nc.scalar.dma_start(
                wl[:, :, :w],
                wlm_d.ap()[:, off:off + w].rearrange("(k p) n -> p k n", p=P))
            wl8s[i] = wl

        early_ctx = ExitStack()
        early = early_ctx.enter_context(tc.tile_pool(name="early", bufs=1))
        xT = early.tile([P, KC, T], BF16)     # x transposed [c, t] * SX
        xT8 = early.tile([P, KC, T], F8)      # fp8 copy for q/k projections

        # ---- phase 1: embedding gather + positional add + transpose ----
        with tc.tile_pool(name="ph1", bufs=1) as ph1, \
             tc.tile_pool(name="ps1", bufs=4, space="PSUM") as ps1:
            idx_ts, xgs, xps = [], [], []
            for t in range(TT):
                idx_t = ph1.tile([P, 1], I32, tag=f"idx{t}")
                nc.sync.dma_start(idx_t[:], idx_d.ap()[t * P:(t + 1) * P, None])
                idx_ts.append(idx_t)
                xp = ph1.tile([P, C], BF16, tag=f"xp{t}")
                nc.scalar.dma_start(xp[:], wpos_d.ap()[t * P:(t + 1) * P, :])
                xps.append(xp)
                xg = ph1.tile([P, C], BF16, tag=f"xg{t}")
                nc.gpsimd.indirect_dma_start(
                    out=xg[:], out_offset=None, in_=wemb_d.ap()[:],
                    in_offset=bass.IndirectOffsetOnAxis(ap=idx_ts[t][:, :1], axis=0))
                xgs.append(xg)
            wq_s = early.tile([P, KC, C], F8)
            wk_s = early.tile([P, KC, C], F8)
            wv_s = early.tile([P, KC, C], BF16)
            for k in range(KC):
                for (w_s, w_d) in ((wq_s, wq_d), (wk_s, wk_d), (wv_s, wv_d)):
                    nc.sync.dma_start(
                        w_s[:, k],
                        w_d.ap()[k * P:(k + 1) * P, :])
            nc.sync.dma_start(blm_s[:], blm_d.ap().rearrange("(o p) -> p o", p=P))
            for t in range(TT):
                xg, xp = xgs[t], xps[t]
                xb = ph1.tile([P, C], BF16, tag=f"xb{t}")
                nc.vector.tensor_add(out=xb[:], in0=xg[:], in1=xp[:])
                for k in range(KC):
                    tp = ps1.tile([P, P], BF16, tag="tp")
                    nc.tensor.transpose(
                        tp[:], xb[:, k * P:(k + 1) * P], ident_bf[:])
                    nc.vector.tensor_copy(xT[:, k, t * P:(t + 1) * P], tp[:])
                    nc.scalar.copy(xT8[:, k, t * P:(t + 1) * P], tp[:])
            # start the W_lm stream once the phase-1 gather traffic has
            # drained (issuing it earlier steals HBM bandwidth from the
            # startup-critical embedding gather)
            for i in range(WPIPE):
                issue_w(i)

        # ---- phase 2: q/k (fp8 DoubleRow) and v (bf16) projections ----
        QSC = SQ / (SX * SW)   # psum q = q*SX*SW -> store q*SQ
        VSC = 1.0 / SX         # psum v = v*SX    -> store v
        with tc.tile_pool(name="ps2", bufs=6, space="PSUM") as ps2:
            for half in range(2):
                for (w_s, dstT) in ((wq_s, qT), (wk_s, kT)):
                    for h in range(KC):
                        pt = ps2.tile([P, 512], F32, tag="qk")
                        for k in range(KD):
                            nc.tensor.matmul(
                                pt[:],
                                w_s[:, 2 * k:2 * k + 2, h * P:(h + 1) * P],
                                xT8[:, 2 * k:2 * k + 2, half * 512:(half + 1) * 512],
                                start=(k == 0), stop=(k == KD - 1), perf_mode=DR)
                        nc.scalar.activation(
                            dstT[:, h, half * 512:(half + 1) * 512], pt[:],
                            IDENT, scale=QSC)
            for s in range(TT):
                for (n0, n1) in ((0, 512), (512, 768)):
                    pt = ps2.tile([P, 512], F32, tag="qk")
                    for k in range(KC):
                        nc.tensor.matmul(
                            pt[:, :n1 - n0],
                            xT[:, k, s * P:(s + 1) * P],
                            wv_s[:, k, n0:n1],
                            start=(k == 0), stop=(k == KC - 1))
                    nc.scalar.activation(
                        v_s[:, s, n0:n1], pt[:, :n1 - n0], IDENT, scale=VSC)
        early_ctx.close()

        pT_ctx = ExitStack()
        pTp = pT_ctx.enter_context(tc.tile_pool(name="pTp", bufs=1))
        # attention probs transposed [s, t], one tile per s-chunk
        pT = [pTp.tile([P, T], BF16, tag=f"pT{s}", name=f"pT{s}")
              for s in range(TT)]

        # zero the strictly-upper (future) blocks of pT that phase-4 matmuls
        # will read but phase 3 never writes
        for s in range(1, TT):
            lo = 0 if s < 4 else 512
            if s * P > lo:
                nc.vector.memset(pT[s][:, lo:s * P].bitcast(mybir.dt.uint16), 0)

        # ---- phase 3: causal attention rows -> pT (fp8 DoubleRow scores) ----
        ESC = SCALE / (SQ * SQ)  # psum scores = score*SQ^2
        with tc.tile_pool(name="ph3", bufs=3) as ph3, \
             tc.tile_pool(name="ps3", bufs=4, space="PSUM") as ps3:
            for t in range(TT):
                L = (t + 1) * P
                srow = ph3.tile([P, T], F32, tag="srow")
                for b0 in range(0, L, 512):
                    n = min(512, L - b0)
                    pt = ps3.tile([P, 512], F32, tag="sc")
                    for k in range(KD):
                        nc.tensor.matmul(
                            pt[:, :n],
                            qT[:, 2 * k:2 * k + 2, t * P:(t + 1) * P],
                            kT[:, 2 * k:2 * k + 2, b0:b0 + n],
                            start=(k == 0), stop=(k == KD - 1), perf_mode=DR)
                    nc.scalar.copy(srow[:, b0:b0 + n], pt[:, :n])
                # causal mask on the diagonal block
                nc.vector.tensor_add(
                    out=srow[:, t * P:(t + 1) * P],
                    in0=srow[:, t * P:(t + 1) * P], in1=cmask[:])
                nmax = ph3.tile([P, 1], F32, tag="nmax")
                nc.vector.tensor_reduce(
                    nmax[:], srow[:, :L], axis=mybir.AxisListType.X,
                    op=mybir.AluOpType.max, negate=True)
                nbias = ph3.tile([P, 1], F32, tag="nbias")
                nc.vector.tensor_scalar_mul(nbias[:], nmax[:], ESC)
                prow = ph3.tile([P, T], BF16, tag="prow")
                rsum = ph3.tile([P, 1], F32, tag="rsum")
                nc.scalar.activation(
                    prow[:, :L], srow[:, :L], mybir.ActivationFunctionType.Exp,
                    bias=nbias[:, :1], scale=ESC, accum_out=rsum[:, :1])
                rinv = ph3.tile([P, 1], F32, tag="rinv")
                nc.vector.reciprocal(rinv[:], rsum[:])
                nc.vector.tensor_scalar_mul(prow[:, :L], prow[:, :L], rinv[:, :1])
                for s in range(t + 1):
                    tp = ps3.tile([P, P], BF16, tag="tp")
                    nc.tensor.transpose(
                        tp[:], prow[:, s * P:(s + 1) * P], ident_bf[:])
                    nc.vector.tensor_copy(pT[s][:, t * P:(t + 1) * P], tp[:])

        # ---- phase 4: att_out = P @ v (bf16) ----
        with tc.tile_pool(name="ps4", bufs=4, space="PSUM") as ps4:
            for h in range(KC):
                for blk in range(2):
                    smax = 4 if blk == 0 else TT
                    pt = ps4.tile([P, 512], F32, tag="av")
                    for s in range(smax):
                        nc.tensor.matmul(
                            pt[:],
                            v_s[:, s, h * P:(h + 1) * P],
                            pT[s][:, blk * 512:(blk + 1) * 512],
                            start=(s == 0), stop=(s == smax - 1))
                    nc.scalar.activation(
                        oT[h // 2][:, h % 2, blk * 512:(blk + 1) * 512], pt[:],
                        IDENT, scale=SO)
                    if blk == 0:
                        nc.vector.tensor_copy(oTe[:, h, :], pt[:, :P])

        # ---- phase 5: LM head ----
        # main pass: fp8 DoubleRow, logitsT[v, t] for t in 128..1023
        # early pass: bf16, logitsE[t, v] for t in 0..127 (host overlays)
        LSC = 1.0 / (SO * SWL)
        ESC5 = 1.0 / SWL
        with tc.tile_pool(name="ps5", bufs=6, space="PSUM") as ps5, \
             tc.tile_pool(name="ps5e", bufs=2, space="PSUM") as ps5e:
            for i, off in enumerate(offs):
                w = min(CHUNK, VH - off)
                wlb = wlbs.pop(i)
                wl = wl8s.pop(i)
                issue_w(i + WPIPE)
                for j in range(w // P):
                    vt = (off + j * P) // P
                    lo = out5.tile([P, T], BF16, tag="lo")
                    pts = [ps5.tile([P, 512], F32, tag="lp", name=f"lp{hf}")
                           for hf in range(2)]
                    for k in range(KD):
                        nc.tensor.matmul(
                            pts[0][:, :384],
                            wl[:, 2 * k:2 * k + 2, j * P:(j + 1) * P],
                            oT[k][:, 0:2, P:512],
                            start=(k == 0), stop=(k == KD - 1), perf_mode=DR)
                        nc.tensor.matmul(
                            pts[1][:],
                            wl[:, 2 * k:2 * k + 2, j * P:(j + 1) * P],
                            oT[k][:, 0:2, 512:1024],
                            start=(k == 0), stop=(k == KD - 1), perf_mode=DR)
                    if with_bias:
                        nc.scalar.activation(
                            lo[:, P:512], pts[0][:, :384], IDENT,
                            bias=blm_s[:, vt:vt + 1], scale=LSC)
                        nc.scalar.activation(
                            lo[:, 512:1024], pts[1][:], IDENT,
                            bias=blm_s[:, vt:vt + 1], scale=LSC)
                    else:
                        nc.vector.tensor_scalar_mul(
                            lo[:, P:512], pts[0][:, :384], LSC)
                        nc.vector.tensor_scalar_mul(
                            lo[:, 512:1024], pts[1][:], LSC)
                    nc.sync.dma_start(
                        out_d.ap()[vt * P:(vt + 1) * P, P:], lo[:, P:])
                # early correction pass for this vocab chunk
                for vs in range(0, w, 512):
                    n = min(512, w - vs)
                    pte = ps5e.tile([P, 512], F32, tag="lpe")
                    for k in range(KC):
                        nc.tensor.matmul(
                            pte[:, :n], oTe[:, k, :], wlb[:, k, vs:vs + n],
                            start=(k == 0), stop=(k == KC - 1))
                    loe = oute5.tile([P, 512], BF16, tag="loe")
                    nc.vector.tensor_scalar_mul(loe[:, :n], pte[:, :n], ESC5)
                    nc.sync.dma_start(
                        oute_d.ap()[:, off + vs:off + vs + n], loe[:, :n])
        pT_ctx.close()
        att_ctx.close()


def _get_nc(with_bias):
    key = ("nc", with_bias)
    if key not in _CACHE:
        _CACHE[key] = _build(with_bias)
    return _CACHE[key]


def _to_f8(x, scale):
    import ml_dtypes
    y = np.clip(np.asarray(x, np.float32) * np.float32(scale), -240.0, 240.0)
    return np.ascontiguousarray(y.astype(ml_dtypes.float8_e4m3))


def _make_in_maps(idx, W_embed, W_pos, Wq, Wk, Wv, W_lm, b_lm):
    import ml_dtypes

    W_embed = np.ascontiguousarray(
        (np.asarray(W_embed, np.float32) * np.float32(SX)).astype(
            ml_dtypes.bfloat16))
    W_pos = np.ascontiguousarray(
        (np.asarray(W_pos, np.float32) * np.float32(SX)).astype(
            ml_dtypes.bfloat16))
    wq8 = _to_f8(Wq, SW)
    wk8 = _to_f8(Wk, SW)
    wvb = np.ascontiguousarray(np.asarray(Wv).astype(ml_dtypes.bfloat16))
    halves_w8 = []
    halves_wb = []
    halves_b = []
    for h in range(2):
        lo = h * VH
        hi = min(VOCAB, lo + VH)
        wlb = np.zeros((C, VH), dtype=ml_dtypes.bfloat16)
        wlb[:, :hi - lo] = (np.asarray(W_lm[:, lo:hi], np.float32)
                            * np.float32(SWL)).astype(ml_dtypes.bfloat16)
        wl8 = np.zeros((C, VH), dtype=ml_dtypes.float8_e4m3)
        wl8[:, :hi - lo] = _to_f8(W_lm[:, lo:hi], SWL)
        bl = np.zeros((VH,), dtype=np.float32)
        bl[:hi - lo] = b_lm[lo:hi]
        halves_w8.append(wl8)
        halves_wb.append(wlb)
        halves_b.append(bl)
    in_maps = []
    for core in range(8):
        b = core >> 1
        h = core & 1
        in_maps.append({
            "idx": np.ascontiguousarray(idx[b], dtype=np.int32),
            "W_embed": W_embed,
            "W_pos": W_pos,
            "Wq": wq8,
            "Wk": wk8,
            "Wv": wvb,
            "W_lm": halves_w8[h],
            "W_lm_bf": halves_wb[h],
            "b_lm": halves_b[h],
        })
    return in_maps


def _run(inputs, trace=False):
    nc = _get_nc(bool(np.any(np.asarray(inputs["b_lm"]))))
    in_maps = _make_in_maps(**inputs)
    res = bass_utils.run_bass_kernel_spmd(
        nc, in_maps, core_ids=list(range(8)), trace=trace)
    B = inputs["idx"].shape[0]
    b_lm = np.asarray(inputs["b_lm"], np.float32)
    out = np.empty((B, T, VOCAB), dtype=np.float32)
    for core in range(8):
        b = core >> 1
        h = core & 1
        lo = h * VH
        hi = min(VOCAB, lo + VH)
        out[b, :, lo:hi] = res.results[core]["logitsT"][:hi - lo, :].astype(
            np.float32).T
        out[b, :P, lo:hi] = res.results[core]["logitsE"][:, :hi - lo].astype(
            np.float32) + b_lm[lo:hi]
    return out, res


def kernel(**inputs):
    out, _ = _run(inputs, trace=False)
    return out
